# revision 2
# baseline (speedup 1.0000x reference)
"""Trainium2 Bass kernel: bidirectional-LSTM final-cell-state encoder.

Problem: 4 independent BasicLSTMCell chains (premise fw/bw, hypothesis fw/bw),
B=1024, T=128, D=300, H=100.  Output [B, 4H] = concat of final cell states.

Strategy
--------
* Data-parallel: batch sharded 8 ways -> 128 rows/core, each core runs all
  4 chains as 2 decoupled pair-chains so recurrences hide each other's
  latency.
* z_t [128b, 400g] accumulated in PSUM from 4 matmuls per run
  (3 pre-transposed x chunks + recurrent h^T chunk, all bf16).
* Gate columns permuted host-side to (i, f, 2j, o) with the j columns
  pre-scaled by 2 so ONE sigmoid covers i, f and j (tanh(j) = 2*sigmoid(2j)-1,
  fixed up by a single fused DVE tensor_scalar); sigmoid(o) is issued
  separately so it never blocks the c-update chain.
* Software-pipelined issue order: each pair's transpose/copy/rec-matmuls are
  emitted at the top of the next iteration, before that pair's chain, so the
  PE FIFO never holds one pair's next-step rec-matmul behind the other
  pair's late tail; x-projection matmuls for t+1 are interleaved per pair.
* h^T for the next step: bf16 PE transpose into dead PSUM columns of the z
  tile, evacuated by one DVE copy per run (contiguous, 2x DVE mode).
"""

import numpy as np

B, T, D, H = 1024, 128, 300, 100
NCORES = 8
BL = B // NCORES          # 128 batch rows per core
G4 = 4 * H                # 400 gate columns
KCH = 128                 # 100 d-rows + 1 ones-row + zero-pad (FWL needs K=128)
TB = 8                    # timesteps per DMA block
FORGET_BIAS = 1.0

_CACHE = {}


def _build_program(n_steps=T):
    from contextlib import ExitStack

    import concourse.mybir as mybir
    import concourse.tile as tile
    from concourse import bacc

    f32 = mybir.dt.float32
    bf16 = mybir.dt.bfloat16
    Sig = mybir.ActivationFunctionType.Sigmoid
    Tanh = mybir.ActivationFunctionType.Tanh
    mult = mybir.AluOpType.mult
    add = mybir.AluOpType.add

    nc = bacc.Bacc(
        "TRN2",
        target_bir_lowering=False,
        debug=False,
        enable_asserts=False,
        num_devices=NCORES,
    )

    xt_p = nc.dram_tensor("xt_p", [T // TB, KCH, TB * 3 * BL], bf16, kind="ExternalInput").ap()
    xt_h = nc.dram_tensor("xt_h", [T // TB, KCH, TB * 3 * BL], bf16, kind="ExternalInput").ap()
    w_all = nc.dram_tensor("w_all", [KCH, 16 * G4], bf16, kind="ExternalInput").ap()
    wh_bf = nc.dram_tensor("wh_bf", [128, 4 * G4], bf16, kind="ExternalInput").ap()
    ident = nc.dram_tensor("ident", [128, 128], bf16, kind="ExternalInput").ap()
    out = nc.dram_tensor("out", [BL, G4], f32, kind="ExternalOutput").ap()

    with tile.TileContext(nc) as tc, ExitStack() as ctx:
        w_sb = nc.alloc_sbuf_tensor("w_sb", [KCH, 16 * G4], bf16).ap()
        wh_sb = nc.alloc_sbuf_tensor("wh_sb", [128, 4 * G4], bf16).ap()
        id_sb = nc.alloc_sbuf_tensor("id_sb", [128, 128], bf16).ap()

        # per-pair state (pair p owns runs 2p, 2p+1); SP holds sigmoid outputs
        # r-major as (i, f, s2j, o) per run
        cP, SP, TJP, TCP, T1P, T2P, HNP, HTP = [], [], [], [], [], [], [], []
        for p in range(2):
            cP.append(nc.alloc_sbuf_tensor(f"c{p}", [BL, 200], f32).ap())
            SP.append(nc.alloc_sbuf_tensor(f"s{p}", [BL, 800], bf16).ap())
            TJP.append(nc.alloc_sbuf_tensor(f"tj{p}", [BL, 200], bf16).ap())
            TCP.append(nc.alloc_sbuf_tensor(f"tc{p}", [BL, 200], bf16).ap())
            T1P.append(nc.alloc_sbuf_tensor(f"t1{p}", [BL, 200], bf16).ap())
            T2P.append(nc.alloc_sbuf_tensor(f"t2{p}", [BL, 200], f32).ap())
            HNP.append(nc.alloc_sbuf_tensor(f"hn{p}", [BL, 200], bf16).ap())
            HTP.append(nc.alloc_sbuf_tensor(f"ht{p}", [128, 256], bf16).ap())

        nc.gpsimd.dma_start(w_sb, w_all)
        nc.gpsimd.dma_start(wh_sb, wh_bf)
        nc.gpsimd.dma_start(id_sb, ident)
        for p in range(2):
            nc.vector.memset(cP[p], 0.0)
            nc.vector.memset(HTP[p], 0.0)

        xt_pools = [
            ctx.enter_context(tc.tile_pool(name=f"xt{s}", bufs=2)) for s in range(4)
        ]
        zpools = [
            ctx.enter_context(tc.tile_pool(name=f"zp{p}", bufs=2, space="PSUM"))
            for p in range(2)
        ]

        # stream s: (dram tensor, reversed?) for runs (p_fw, p_bw, h_fw, h_bw)
        streams = [(xt_p, False), (xt_p, True), (xt_h, False), (xt_h, True)]
        cur = [None] * 4

        def kick_dma(s_step, tag):
            for si, (dram, rev) in enumerate(streams):
                tl = xt_pools[si].tile(
                    [KCH, TB * 3 * 128], bf16, tag=f"x{si}", name=f"x{si}_{tag}"
                )
                blk = s_step // TB
                nblk = (T // TB - 1 - blk) if rev else blk
                nc.gpsimd.dma_start(tl[:, :], dram[nblk])
                cur[si] = tl

        def x_mms(p, s):
            """x-projection matmuls for pair p, step s (into z tile zS[p])."""
            for r in range(2):
                u = 2 * p + r
                rev = streams[u][1]
                tq = (TB - 1 - s % TB) if rev else (s % TB)
                zc = znext[p][:, r * 512 : r * 512 + G4]
                tl = cur[u]
                for k in range(3):
                    nc.tensor.matmul(
                        zc,
                        tl[:, (tq * 3 + k) * 128 : (tq * 3 + k + 1) * 128],
                        w_sb[:, (u * 4 + k) * G4 : (u * 4 + k + 1) * G4],
                        start=(k == 0),
                        stop=(s == 0 and k == 2),
                    )

        # ---- prologue: block 0 DMA + step-0 x matmuls ----
        kick_dma(0, "pro")
        znext = [
            zpools[p].tile([BL, 1024], f32, tag=f"z{p}", name=f"z{p}_0")
            for p in range(2)
        ]
        for p in range(2):
            x_mms(p, 0)
        zcur = znext
        zprev = [None, None]

        for tt in range(n_steps):
            last = tt == n_steps - 1
            z3 = [zcur[p][:, :].rearrange("b (r c) -> b r c", r=2) for p in range(2)]
            sp3 = [SP[p][:, :].rearrange("b (r c) -> b r c", r=2) for p in range(2)]
            tj3 = [TJP[p][:, :].rearrange("b (r c) -> b r c", r=2) for p in range(2)]
            c3 = [cP[p][:, :].rearrange("b (r c) -> b r c", r=2) for p in range(2)]
            t13 = [T1P[p][:, :].rearrange("b (r c) -> b r c", r=2) for p in range(2)]
            t23 = [T2P[p][:, :].rearrange("b (r c) -> b r c", r=2) for p in range(2)]

            if not last:
                znext = [
                    zpools[p].tile([BL, 1024], f32, tag=f"z{p}", name=f"z{p}_{tt+1}")
                    for p in range(2)
                ]

            for p in range(2):
                if tt > 0:
                    zq = zprev[p]
                    for r in range(2):
                        # h(t-1)^T via bf16 PE transpose into dead z cols
                        nc.tensor.transpose(
                            zq[0:H, r * 512 : r * 512 + 64].bitcast(bf16),
                            HNP[p][:, r * 100 : r * 100 + 100],
                            id_sb,
                        )
                    for r in range(2):
                        nc.vector.tensor_copy(
                            HTP[p][0:H, r * 128 : (r + 1) * 128],
                            zq[0:H, r * 512 : r * 512 + 64].bitcast(bf16),
                        )
                    for r in range(2):
                        u = 2 * p + r
                        nc.tensor.matmul(
                            zcur[p][:, r * 512 : r * 512 + G4],
                            HTP[p][:, r * 128 : (r + 1) * 128],
                            wh_sb[:, u * G4 : (u + 1) * G4],
                            start=False,
                            stop=True,
                        )
                # sigmoid over (i, f, 2j) for both runs of this pair
                nc.scalar.activation(sp3[p][:, :, 0:300], z3[p][:, :, 0:300], Sig)
                # prefetch next step's x matmuls for this pair
                if not last:
                    if p == 0 and (tt + 1) % TB == 0:
                        kick_dma(tt + 1, tt + 1)
                    x_mms(p, tt + 1)
                if p == 0:
                    # pair-0 DVE chain (t2 first: needs only sigmoid(f))
                    nc.vector.tensor_tensor(
                        t23[0], sp3[0][:, :, 100:200], c3[0], mult
                    )
                    nc.vector.tensor_scalar(
                        out=tj3[0], in0=sp3[0][:, :, 200:300],
                        scalar1=2.0, op0=mult, scalar2=-1.0, op1=add,
                    )
                    nc.vector.tensor_tensor(
                        t13[0], sp3[0][:, :, 0:100], tj3[0], mult
                    )
                    nc.vector.tensor_tensor(cP[0], T1P[0], T2P[0], add)

            # pair-0 finishers / pair-1 DVE chain
            if not last:
                nc.scalar.activation(sp3[0][:, :, 300:400], z3[0][:, :, 300:400], Sig)
                nc.scalar.activation(TCP[0], cP[0], Tanh)
            nc.vector.tensor_tensor(t23[1], sp3[1][:, :, 100:200], c3[1], mult)
            nc.vector.tensor_scalar(
                out=tj3[1], in0=sp3[1][:, :, 200:300],
                scalar1=2.0, op0=mult, scalar2=-1.0, op1=add,
            )
            nc.vector.tensor_tensor(t13[1], sp3[1][:, :, 0:100], tj3[1], mult)
            nc.vector.tensor_tensor(cP[1], T1P[1], T2P[1], add)
            if not last:
                nc.scalar.activation(sp3[1][:, :, 300:400], z3[1][:, :, 300:400], Sig)
                nc.scalar.activation(TCP[1], cP[1], Tanh)
                for p in range(2):
                    for r in range(2):
                        nc.vector.tensor_tensor(
                            HNP[p][:, r * 100 : (r + 1) * 100],
                            TCP[p][:, r * 100 : (r + 1) * 100],
                            SP[p][:, r * 400 + 300 : r * 400 + 400],
                            mult,
                        )
            else:
                for p in range(2):
                    nc.sync.dma_start(out[:, p * 200 : (p + 1) * 200], cP[p])

            zprev = zcur
            zcur = znext

    nc.compile()
    return nc


def _prep_xt(x_slice):
    """[BL, T, D] fp32 -> [T//TB, 101, TB*3*BL] bf16 block-major tiles.

    tile[n, p, (tq, j, b)] = x[b, n*TB+tq, j*100+p] for p<100; p=100 is the
    baked-in ones row (bias trick).  Each DMA block is a plain 2D copy with
    TB*3*BL*2 contiguous bytes per partition.
    """
    import ml_dtypes

    a = x_slice.transpose(1, 2, 0).reshape(T // TB, TB, 3, 100, BL)
    a = a.transpose(0, 3, 1, 2, 4)  # [n, p, tq, j, b]
    outp = np.zeros((T // TB, KCH, TB, 3, BL), ml_dtypes.bfloat16)
    outp[:, :100] = a.astype(ml_dtypes.bfloat16)
    outp[:, 100] = 1.0
    return outp.reshape(T // TB, KCH, TB * 3 * BL)


def _prep_weights(Ws, bs):
    """Pack 4 runs' [D+H, 4H] weights into [101, 16*400] chunk blocks.

    Gate columns permuted (i,j,f,o) -> (i,f,2j,o) with the j block scaled by
    2 (tanh(j) = 2*sigmoid(2j)-1 on device); chunk-2's row 100 carries
    b_perm + the +1.0 forget bias (paired with the baked-in x ones-row).
    Also emits the recurrent rows (300:400) as bf16 [100, 4*400].
    """
    import ml_dtypes

    perm = np.concatenate(
        [np.arange(0, 100), np.arange(200, 300), np.arange(100, 200), np.arange(300, 400)]
    )
    w_all = np.zeros((KCH, 16 * G4), ml_dtypes.bfloat16)
    wh_bf = np.zeros((128, 4 * G4), ml_dtypes.bfloat16)
    for u in range(4):
        Wp = np.asarray(Ws[u], np.float32)[:, perm].copy()  # [400, 400]
        Wp[:, 200:300] *= 2.0
        bp = np.asarray(bs[u], np.float32)[perm].copy()
        bp[200:300] *= 2.0
        for k in range(3):
            blk = w_all[:, (u * 4 + k) * G4 : (u * 4 + k + 1) * G4]
            blk[0:100] = Wp[k * 100 : (k + 1) * 100].astype(ml_dtypes.bfloat16)
        bias_row = bp.copy()
        bias_row[100:200] += FORGET_BIAS
        w_all[100, (u * 4 + 2) * G4 : (u * 4 + 3) * G4] = bias_row.astype(
            ml_dtypes.bfloat16
        )
        wh_bf[0:H, u * G4 : (u + 1) * G4] = Wp[300:400].astype(ml_dtypes.bfloat16)
    return w_all, wh_bf


def kernel(premises, hypotheses, Wp_fw, bp_fw, Wp_bw, bp_bw, Wh_fw, bh_fw, Wh_bw, bh_bw):
    from concourse.bass_utils import run_bass_kernel_spmd

    if "nc" not in _CACHE:
        _CACHE["nc"] = _build_program()
    nc = _CACHE["nc"]

    w_all, wh_bf = _prep_weights(
        [Wp_fw, Wp_bw, Wh_fw, Wh_bw], [bp_fw, bp_bw, bh_fw, bh_bw]
    )
    import ml_dtypes

    ident = np.eye(128, dtype=ml_dtypes.bfloat16)

    in_maps = []
    for c in range(NCORES):
        sl = slice(c * BL, (c + 1) * BL)
        in_maps.append(
            {
                "xt_p": _prep_xt(np.asarray(premises[sl], np.float32)),
                "xt_h": _prep_xt(np.asarray(hypotheses[sl], np.float32)),
                "w_all": w_all,
                "wh_bf": wh_bf,
                "ident": ident,
            }
        )

    res = run_bass_kernel_spmd(nc, in_maps, core_ids=list(range(NCORES)))
    out = np.concatenate([r["out"] for r in res.results], axis=0)
    # columns are (c_pf, c_pb, c_hf, c_hb) in run order already
    return out
